# revision 1
# baseline (speedup 1.0000x reference)
"""Multi-head attention TRN2 kernel.

Sharding: 8 cores = 4 batches x 2 head-groups (Megatron tensor parallel over
the 16 heads: Wq/Wk/Wv column-sharded, Wo row-sharded; partial outputs summed
per batch on the host).

Per-core device kernel (batch b, head-group hg -> heads 8hg..8hg+8):
  qT = WqT.T @ xqT            [512, 2048]  (d-major: heads pairwise stacked)
  kT = WkT.T @ xkT            [512, 2048]
  v  = xvT.T  @ WvT           [2048, 512]  + ones column per head
  per (s_tile 512, head-pair, t_chunk 128):
     scoresT[t,s] = kT_h.T @ qT_h     (K=64, heads row-packed 0-63/64-127)
     exp on ACT from PSUM ([128,1024] = both heads), scale=1/sqrt(64)
     ctxT_aug[65,s] += v_aug.T @ expT (ones row accumulates softmax sums)
  normalize: ctxT *= 1/sums (bcast via K=1 matmul), pack into ctxT [512,2048]
  out_partial = ctxT.T @ WoT  [2048, 1024]
"""

import os
import sys
from contextlib import ExitStack

for _p in ("/opt/trn_rl_repo", "/root/.axon_site/_ro/trn_rl_repo"):
    if os.path.isdir(_p) and _p not in sys.path:
        sys.path.insert(0, _p)
        break

import numpy as np

import concourse.bass as bass
import concourse.bacc as bacc
import concourse.mybir as mybir
import concourse.tile as tile

B, S, E, H, D = 4, 2048, 1024, 16, 64
HG = 2          # head groups (tensor-parallel factor)
DH = E // HG    # 512 dims per head group (8 heads)
HPG = H // HG   # 8 heads per group
NCORES = B * HG

F32 = mybir.dt.float32
# matmul operand dtype: float32r streams at ~1 cycle/row (vs 4 for float32)
_MM_DT_NAME = os.environ.get("BASS_MHA_MM_DT", "float32r")
MM_DT = getattr(mybir.dt, _MM_DT_NAME)

SCALE = 1.0 / np.sqrt(D)


def _mm(nc, out, lhsT, rhs, start, stop):
    if lhsT.dtype != MM_DT:
        lhsT = lhsT.bitcast(MM_DT)
    if rhs.dtype != MM_DT:
        rhs = rhs.bitcast(MM_DT)
    nc.tensor.matmul(out, lhsT=lhsT, rhs=rhs, start=start, stop=stop)


def build_nc():
    nc = bacc.Bacc()
    xqT = nc.declare_dram_parameter("xqT", [E, S], MM_DT, isOutput=False)
    xkT = nc.declare_dram_parameter("xkT", [E, S], MM_DT, isOutput=False)
    xvT = nc.declare_dram_parameter("xvT", [E, S], MM_DT, isOutput=False)
    wqT = nc.declare_dram_parameter("wqT", [E, DH], MM_DT, isOutput=False)
    wkT = nc.declare_dram_parameter("wkT", [E, DH], MM_DT, isOutput=False)
    wvT = nc.declare_dram_parameter("wvT", [E, DH], MM_DT, isOutput=False)
    woT = nc.declare_dram_parameter("woT", [DH, E], MM_DT, isOutput=False)
    out = nc.declare_dram_parameter("out", [S, E], F32, isOutput=True)

    with (
        nc.allow_low_precision(reason="float32r matmul operands"),
        tile.TileContext(nc) as tc,
        ExitStack() as ctx,
    ):
        _emit(ctx, tc, xqT, xkT, xvT, wqT, wkT, wvT, woT, out)
    nc.compile()
    return nc


def _final_proj(nc, fps, osb, ctxT_sb, wo_sb, out, s0):
    DC = DH // 128
    for si in range(4):
        r0 = s0 + 128 * si
        o_sb = osb.tile([128, E], F32, tag="osb", name=f"osb_{r0}")
        for et in range(2):
            fp = fps.tile([128, 512], F32, tag="fp", name=f"fp_{r0}_{et}")
            for c in range(DC):
                _mm(
                    nc,
                    fp,
                    ctxT_sb[:, c, r0 : r0 + 128],
                    wo_sb[:, c, 512 * et : 512 * (et + 1)],
                    start=(c == 0),
                    stop=(c == DC - 1),
                )
            nc.vector.tensor_copy(o_sb[:, 512 * et : 512 * (et + 1)], fp)
        nc.sync.dma_start(out=out[r0 : r0 + 128, :], in_=o_sb)


def _emit(ctx, tc, xqT, xkT, xvT, wqT, wkT, wvT, woT, out):
    nc = tc.nc
    EC = E // 128    # 8 contraction chunks for projections
    DC = DH // 128   # 4 d-chunks of the head group
    TC = S // 128    # 16 t chunks
    ST = S // 512    # 4 s tiles
    DA = D + 1       # 65: head dim + ones column

    # ---- persistent tensors ----------------------------------------------
    big = ctx.enter_context(tc.tile_pool(name="big", bufs=1))
    # qT/kT: [p, c, s] with global d = 128*c + p  (head 2c on p 0-63, 2c+1 on 64-127)
    qT_sb = big.tile([128, DC, S], MM_DT, tag="qT")
    kT_sb = big.tile([128, DC, S], MM_DT, tag="kT")
    # v_aug: [t%128, t_chunk, head, 65]; col 64 is the ones column
    v_sb = big.tile([128, TC, HPG, DA], MM_DT, tag="v")
    # ctxT: [p, c, s], hd = 128*c + p
    ctxT_sb = big.tile([128, DC, S], MM_DT, tag="ctx")
    ones_col = big.tile([128, 1], F32, tag="ones_col")
    nc.vector.memset(ones_col, 1.0)
    wo_sb = big.tile([128, DC, E], MM_DT, tag="wo")
    for a in range(DC):
        nc.scalar.dma_start(
            out=wo_sb[:, a, :], in_=woT[128 * a : 128 * (a + 1), :]
        )
    for t in range(TC):
        nc.vector.tensor_copy(
            v_sb[:, t, :, D : D + 1],
            ones_col.to_broadcast((128, HPG)).rearrange("p (h o) -> p h o", o=1),
        )

    # ---- stage B: qT / kT projections ------------------------------------
    # qT[d, s] accumulated over e: lhsT = WqT block [e, d], rhs = xqT [e, s]
    with (
        tc.tile_pool(name="wqk", bufs=1) as wqk,
        tc.tile_pool(name="xin", bufs=5) as xin,
        tc.tile_pool(name="bps", bufs=2, space="PSUM") as bps,
    ):
        wq_sb = wqk.tile([128, EC, DH], MM_DT, tag="wq")
        wk_sb = wqk.tile([128, EC, DH], MM_DT, tag="wk")
        for e in range(EC):
            nc.sync.dma_start(
                out=wq_sb[:, e, :], in_=wqT[128 * e : 128 * (e + 1), :]
            )
            nc.scalar.dma_start(
                out=wk_sb[:, e, :], in_=wkT[128 * e : 128 * (e + 1), :]
            )
        for x_dram, w_sb, dst in ((xqT, wq_sb, qT_sb), (xkT, wk_sb, kT_sb)):
            for sh in range(2):  # s halves of 1024
                ps = []
                for dc in range(DC):
                    ps.append(bps.tile([128, 1024], F32, tag=f"pb{dc}", bufs=1, name=f"pb{dc}_{sh}"))
                for e in range(EC):
                    xtr = xin.tile([128, 1024], MM_DT, tag="xt")
                    eng = nc.sync if x_dram is xqT else nc.scalar
                    eng.dma_start(
                        out=xtr,
                        in_=x_dram[128 * e : 128 * (e + 1), 1024 * sh : 1024 * (sh + 1)],
                    )
                    for dc in range(DC):
                        lhs = w_sb[:, e, 128 * dc : 128 * (dc + 1)]
                        for sq in range(2):
                            _mm(
                                nc,
                                ps[dc][:, 512 * sq : 512 * (sq + 1)],
                                lhs,
                                xtr[:, 512 * sq : 512 * (sq + 1)],
                                start=(e == 0),
                                stop=(e == EC - 1),
                            )
                for dc in range(DC):
                    nc.vector.tensor_copy(
                        dst[:, dc, 1024 * sh : 1024 * (sh + 1)], ps[dc]
                    )

    # ---- stage C: v projection -------------------------------------------
    # v[t, d] accumulated over e: lhsT = xvT block [e, t], rhs = WvT [e, d]
    with (
        tc.tile_pool(name="wvp", bufs=1) as wvp,
        tc.tile_pool(name="xvin", bufs=5) as xvin,
        tc.tile_pool(name="cps", bufs=1, space="PSUM") as cps,
    ):
        wv_sb = wvp.tile([128, EC, DH], MM_DT, tag="wv")
        for e in range(EC):
            nc.gpsimd.dma_start(
                out=wv_sb[:, e, :], in_=wvT[128 * e : 128 * (e + 1), :]
            )
        for th in range(2):  # t halves of 1024
            pv = []
            for tt in range(8):
                pv.append(cps.tile([128, DH], F32, tag=f"pv{tt}", bufs=1, name=f"pv{th}_{tt}"))
            for e in range(EC):
                xtr = xvin.tile([128, 1024], MM_DT, tag="xvt")
                nc.gpsimd.dma_start(
                    out=xtr,
                    in_=xvT[128 * e : 128 * (e + 1), 1024 * th : 1024 * (th + 1)],
                )
                for tt in range(8):
                    _mm(
                        nc,
                        pv[tt],
                        xtr[:, 128 * tt : 128 * (tt + 1)],
                        wv_sb[:, e, :],
                        start=(e == 0),
                        stop=(e == EC - 1),
                    )
            for tt in range(8):
                t = 8 * th + tt
                # strided copy into per-head layout [128, 8, 64]
                nc.vector.tensor_copy(
                    v_sb[:, t, :, 0:D],
                    pv[tt].rearrange("p (h d) -> p h d", h=HPG),
                )

    # ---- stage D/E: attention + output projection ------------------------
    with (
        tc.tile_pool(name="ex", bufs=4) as expool,
        tc.tile_pool(name="small", bufs=3) as small,
        tc.tile_pool(name="osb", bufs=3) as osb,
        tc.tile_pool(name="dps", bufs=2, space="PSUM") as dps,
        tc.tile_pool(name="cxps", bufs=3, space="PSUM") as cxps,
        tc.tile_pool(name="fps", bufs=1, space="PSUM") as fps,
    ):
        for st in range(ST):
            s0 = 512 * st
            for c in range(DC):  # head pair (2c, 2c+1)
                cx = [cxps.tile([DA, 512], F32, tag="cx", name=f"cx{st}_{c}_{j2}") for j2 in range(2)]
                for t in range(TC):
                    sc = dps.tile([128, 1024], F32, tag="sc")
                    for j in range(2):
                        _mm(
                            nc,
                            sc[:, 512 * j : 512 * (j + 1)],
                            kT_sb[64 * j : 64 * (j + 1), c, 128 * t : 128 * (t + 1)],
                            qT_sb[64 * j : 64 * (j + 1), c, s0 : s0 + 512],
                            start=True,
                            stop=True,
                        )
                    ex = expool.tile([128, 1024], MM_DT, tag="ex")
                    nc.scalar.activation(
                        out=ex,
                        in_=sc,
                        func=mybir.ActivationFunctionType.Exp,
                        scale=float(SCALE),
                    )
                    for j in range(2):
                        _mm(
                            nc,
                            cx[j],
                            v_sb[:, t, 2 * c + j, :],
                            ex[:, 512 * j : 512 * (j + 1)],
                            start=(t == 0),
                            stop=(t == TC - 1),
                        )
                # free cx psum slots fast, normalize from SBUF off-path
                for j in range(2):
                    cxs = small.tile([DA, 512], F32, tag="cxs", name=f"cxs{st}_{c}_{j}")
                    nc.vector.tensor_copy(cxs, cx[j])
                    rec = small.tile([1, 512], F32, tag="rec")
                    nc.vector.reciprocal(rec, cxs[D : D + 1, :])
                    bc_sb = small.tile([64, 512], F32, tag="bcsb")
                    nc.gpsimd.partition_broadcast(bc_sb, rec)
                    nc.vector.tensor_mul(
                        ctxT_sb[64 * j : 64 * (j + 1), c, s0 : s0 + 512],
                        cxs[0:D, :],
                        bc_sb,
                    )
            # output projection for the PREVIOUS s-tile (hides the norm chain)
            if st > 0:
                _final_proj(nc, fps, osb, ctxT_sb, wo_sb, out, 512 * (st - 1))
        _final_proj(nc, fps, osb, ctxT_sb, wo_sb, out, 512 * (ST - 1))


_BUILT = {}


def _get_nc():
    if "nc" not in _BUILT:
        _BUILT["nc"] = build_nc()
    return _BUILT["nc"]


def make_in_maps(query, key, value, Wq, Wk, Wv, Wo):
    ndt = mybir.dt.np(MM_DT)
    query = np.asarray(query, np.float32).astype(ndt)
    key = np.asarray(key, np.float32).astype(ndt)
    value = np.asarray(value, np.float32).astype(ndt)
    Wq = np.asarray(Wq, np.float32).astype(ndt)
    Wk = np.asarray(Wk, np.float32).astype(ndt)
    Wv = np.asarray(Wv, np.float32).astype(ndt)
    Wo = np.asarray(Wo, np.float32).astype(ndt)

    xqT = [np.ascontiguousarray(query[b].T) for b in range(B)]
    xkT = [np.ascontiguousarray(key[b].T) for b in range(B)]
    xvT = [np.ascontiguousarray(value[b].T) for b in range(B)]
    wqT = [np.ascontiguousarray(Wq[DH * g : DH * (g + 1), :].T) for g in range(HG)]
    wkT = [np.ascontiguousarray(Wk[DH * g : DH * (g + 1), :].T) for g in range(HG)]
    wvT = [np.ascontiguousarray(Wv[DH * g : DH * (g + 1), :].T) for g in range(HG)]
    woT = [np.ascontiguousarray(Wo[:, DH * g : DH * (g + 1)].T) for g in range(HG)]

    in_maps = []
    for core in range(NCORES):
        b, g = core // HG, core % HG
        in_maps.append(
            {
                "xqT": xqT[b],
                "xkT": xkT[b],
                "xvT": xvT[b],
                "wqT": wqT[g],
                "wkT": wkT[g],
                "wvT": wvT[g],
                "woT": woT[g],
            }
        )
    return in_maps


def assemble(core_outs):
    out = np.empty((B, S, E), np.float32)
    for b in range(B):
        out[b] = core_outs[HG * b]
        for g in range(1, HG):
            out[b] += core_outs[HG * b + g]
    return out


def kernel(query, key, value, Wq, Wk, Wv, Wo):
    from concourse.bass_utils import run_bass_kernel_spmd

    nc = _get_nc()
    in_maps = make_in_maps(query, key, value, Wq, Wk, Wv, Wo)
    res = run_bass_kernel_spmd(nc, in_maps, list(range(NCORES)))
    return assemble([r["out"] for r in res.results])



# revision 12
# speedup vs baseline: 1.3057x; 1.3057x over previous
"""Multi-head attention TRN2 kernel (v2, bf16 software-pipelined).

Sharding: 8 cores = 4 batches x 2 head-groups (Megatron tensor parallel over
the 16 heads: Wq/Wk/Wv column-sharded, Wo row-sharded; partial outputs summed
per batch on the host).

Per-core schedule (batch b, head-group g -> 8 local heads, 4 head-pairs c):
  prologue: kT(c=0) + qT(st=0,c=0) projections; v/kT(c>0)/qT interleave later
  main loop over (st, c): 16 t-chunks:
     scoresT[t,s] pair = kT_h.T @ qT_h      (K=64, j-pair co-executes on PE)
     ex = exp(scores/8) on ACT (PSUM->SBUF bf16)
     cx[65,s] += v_aug.T @ ex               (ones column -> softmax sums)
  producer mms (v proj, later kT/qT, out-proj) pumped into PE slack so the
  ACT engine (exp, ~283us total) stays saturated; normalize on DVE/Pool.
  out partial = ctxT.T @ WoT accumulated per 128-row chunk, DMA'd out.
"""

import os
import sys
from collections import deque
from contextlib import ExitStack

for _p in ("/opt/trn_rl_repo", "/root/.axon_site/_ro/trn_rl_repo"):
    if os.path.isdir(_p) and _p not in sys.path:
        sys.path.insert(0, _p)
        break

import numpy as np

import concourse.bass as bass
import concourse.bacc as bacc
import concourse.mybir as mybir
import concourse.tile as tile

B, S, E, H, D = 4, 2048, 1024, 16, 64
HG = 2          # head groups (tensor-parallel factor)
DH = E // HG    # 512 dims per head group (8 heads)
HPG = H // HG   # 8 heads per group
NCORES = B * HG

EC = E // 128   # 8 contraction chunks for projections
DC = DH // 128  # 4 d-chunks (head pairs)
TC = S // 128   # 16 t chunks
ST = S // 512   # 4 s tiles
SQ = S // 512   # 4 column blocks for projections
DA = D + 1      # 65: head dim + ones column

F32 = mybir.dt.float32
MM_DT = mybir.dt.bfloat16
SCALE = 1.0 / np.sqrt(D)


def build_nc():
    nc = bacc.Bacc()
    xqT = nc.declare_dram_parameter("xqT", [E, S], MM_DT, isOutput=False)
    xkT = nc.declare_dram_parameter("xkT", [E, S], MM_DT, isOutput=False)
    xvT = nc.declare_dram_parameter("xvT", [E, S], MM_DT, isOutput=False)
    wqT = nc.declare_dram_parameter("wqT", [E, DH], MM_DT, isOutput=False)
    wkT = nc.declare_dram_parameter("wkT", [E, DH], MM_DT, isOutput=False)
    wvT = nc.declare_dram_parameter("wvT", [E, DH], MM_DT, isOutput=False)
    woT = nc.declare_dram_parameter("woT", [DH, E], MM_DT, isOutput=False)
    out = nc.declare_dram_parameter("out", [S, E], F32, isOutput=True)

    with (
        nc.allow_low_precision(reason="bf16 matmul operands"),
        tile.TileContext(nc) as tc,
        ExitStack() as ctx,
    ):
        _emit(ctx, tc, xqT, xkT, xvT, wqT, wkT, wvT, woT, out)
    nc.compile()
    return nc


def _emit(ctx, tc, xqT, xkT, xvT, wqT, wkT, wvT, woT, out):
    nc = tc.nc

    big = ctx.enter_context(tc.tile_pool(name="big", bufs=1))
    # qT/kT/ctxT: [p, c, s] with local dim ld = 128*c + p
    # (head 2c on partitions 0-63, head 2c+1 on 64-127)
    qT_sb = big.tile([128, DC, S], MM_DT, tag="qT")
    kT_sb = big.tile([128, DC, S], MM_DT, tag="kT")
    ctxT_sb = big.tile([128, DC, S], MM_DT, tag="ctx")
    # v_aug: [t%128, t_chunk, head, 65]; col 64 is the ones column
    v_sb = big.tile([128, TC, HPG, DA], MM_DT, tag="v")
    wq_sb = big.tile([128, EC, DH], MM_DT, tag="wq")
    wk_sb = big.tile([128, EC, DH], MM_DT, tag="wk")
    wv_sb = big.tile([128, EC, DH], MM_DT, tag="wv")
    wo_sb = big.tile([128, DC, E], MM_DT, tag="wo")
    xk_st = big.tile([128, EC, S], MM_DT, tag="xk")
    xv_st = big.tile([128, EC, S], MM_DT, tag="xv")
    ones = big.tile([128, 1], MM_DT, tag="ones")
    nc.vector.memset(ones, 1.0)
    nc.vector.tensor_copy(
        v_sb[:, :, :, D : D + 1],
        ones.to_broadcast((128, TC * HPG)).rearrange(
            "p (t h o) -> p t h o", t=TC, h=HPG
        ),
    )

    xqp = ctx.enter_context(tc.tile_pool(name="xqp", bufs=2))
    expool = ctx.enter_context(tc.tile_pool(name="ex", bufs=6))
    osb = ctx.enter_context(tc.tile_pool(name="osb", bufs=2))
    small = ctx.enter_context(tc.tile_pool(name="small", bufs=2))
    scps = ctx.enter_context(tc.tile_pool(name="scps", bufs=2, space="PSUM"))
    cxps = ctx.enter_context(tc.tile_pool(name="cxps", bufs=2, space="PSUM"))
    accps = ctx.enter_context(tc.tile_pool(name="accps", bufs=2, space="PSUM"))

    # ---- DMA helpers: alternate the two DMA-capable idle engines ----------
    dma_state = [0]

    def dma(out_ap, in_ap):
        eng = nc.sync if dma_state[0] % 2 == 0 else nc.gpsimd
        dma_state[0] += 1
        eng.dma_start(out=out_ap, in_=in_ap)

    # priority-ordered loads: what the prologue needs first
    for e in range(EC):
        dma(wk_sb[:, e, :], wkT[128 * e : 128 * (e + 1), :])
    for e in range(EC):
        dma(xk_st[:, e, 0:512], xkT[128 * e : 128 * (e + 1), 0:512])
    for e in range(EC):
        dma(wq_sb[:, e, :], wqT[128 * e : 128 * (e + 1), :])
    xq_tiles = {}
    xq_tiles[0] = xqp.tile([128, EC, 512], MM_DT, tag="xq", name="xq_st0")
    for e in range(EC):
        dma(xq_tiles[0][:, e, :], xqT[128 * e : 128 * (e + 1), 0:512])
    for sq in range(1, SQ):
        for e in range(EC):
            dma(
                xk_st[:, e, 512 * sq : 512 * (sq + 1)],
                xkT[128 * e : 128 * (e + 1), 512 * sq : 512 * (sq + 1)],
            )
    for e in range(EC):
        dma(wv_sb[:, e, :], wvT[128 * e : 128 * (e + 1), :])
    for sq in range(SQ):
        for e in range(EC):
            dma(
                xv_st[:, e, 512 * sq : 512 * (sq + 1)],
                xvT[128 * e : 128 * (e + 1), 512 * sq : 512 * (sq + 1)],
            )
    for a in range(DC):
        dma(wo_sb[:, a, :], woT[128 * a : 128 * (a + 1), :])

    # ---- producer generators (yield after each matmul) --------------------
    def kT_gen(c):
        for sq in range(SQ):
            acc = accps.tile([128, 512], F32, tag="acc", name=f"kacc_{c}_{sq}")
            for e in range(EC):
                nc.tensor.matmul(
                    acc,
                    lhsT=wk_sb[:, e, 128 * c : 128 * (c + 1)],
                    rhs=xk_st[:, e, 512 * sq : 512 * (sq + 1)],
                    start=(e == 0),
                    stop=(e == EC - 1),
                )
                yield
            nc.vector.tensor_copy(kT_sb[:, c, 512 * sq : 512 * (sq + 1)], acc)

    def v_gen(tt):
        acc = accps.tile([128, 512], F32, tag="acc", name=f"vacc_{tt}")
        for e in range(EC):
            nc.tensor.matmul(
                acc,
                lhsT=xv_st[:, e, 128 * tt : 128 * (tt + 1)],
                rhs=wv_sb[:, e, :],
                start=(e == 0),
                stop=(e == EC - 1),
            )
            yield
        nc.vector.tensor_copy(
            v_sb[:, tt, :, 0:D], acc.rearrange("p (h d) -> p h d", h=HPG)
        )

    def qT_gen(st, c):
        xq = xq_tiles[st]
        acc = accps.tile([128, 512], F32, tag="acc", name=f"qacc_{st}_{c}")
        for e in range(EC):
            nc.tensor.matmul(
                acc,
                lhsT=wq_sb[:, e, 128 * c : 128 * (c + 1)],
                rhs=xq[:, e, :],
                start=(e == 0),
                stop=(e == EC - 1),
            )
            yield
        nc.vector.tensor_copy(qT_sb[:, c, 512 * st : 512 * (st + 1)], acc)
        if c == DC - 1 and st + 1 < ST:
            # stage next s-tile's xq chunks
            nxt = xqp.tile([128, EC, 512], MM_DT, tag="xq", name=f"xq_st{st+1}")
            xq_tiles[st + 1] = nxt
            for e in range(EC):
                dma(
                    nxt[:, e, :],
                    xqT[128 * e : 128 * (e + 1), 512 * (st + 1) : 512 * (st + 2)],
                )

    def fp_gen(st, si, o_tile):
        r0 = 512 * st + 128 * si
        for et in range(2):
            fp = accps.tile([128, 512], F32, tag="acc", name=f"fp_{r0}_{et}")
            for cc in range(DC):
                nc.tensor.matmul(
                    fp,
                    lhsT=ctxT_sb[:, cc, r0 : r0 + 128],
                    rhs=wo_sb[:, cc, 512 * et : 512 * (et + 1)],
                    start=(cc == 0),
                    stop=(cc == DC - 1),
                )
                yield
            nc.vector.tensor_copy(o_tile[:, 512 * et : 512 * (et + 1)], fp)
        for p0 in range(0, 128, 32):
            dma(out[r0 + p0 : r0 + p0 + 32, :], o_tile[p0 : p0 + 32, :])

    # producer queue machinery: (key, generator) FIFO with forced drains
    producers = deque()
    done_keys = set()
    cur = [None, None]  # key, generator

    def _finish_cur():
        done_keys.add(cur[0])
        cur[0] = cur[1] = None

    def pump(n):
        emitted = 0
        while emitted < n:
            if cur[1] is None:
                if not producers:
                    return
                cur[0], cur[1] = producers.popleft()
            try:
                next(cur[1])
                emitted += 1
            except StopIteration:
                _finish_cur()

    def pump_until(key):
        while key not in done_keys:
            if cur[1] is None:
                if not producers:
                    raise RuntimeError(f"producer underflow waiting for {key}")
                cur[0], cur[1] = producers.popleft()
            try:
                while True:
                    next(cur[1])
            except StopIteration:
                _finish_cur()

    # ---- prologue ---------------------------------------------------------
    for _ in kT_gen(0):
        pass
    for _ in qT_gen(0, 0):
        pass
    done_keys.add(("kT", 0))
    done_keys.add(("qT", 0, 0))

    # initial producer order: v first (needed by cxmm), then kT/qT for the
    # upcoming head-pairs / s-tiles
    for tt in range(8):
        producers.append((("v", tt), v_gen(tt)))
    producers.append((("qT", 0, 1), qT_gen(0, 1)))
    producers.append((("kT", 1), kT_gen(1)))
    for tt in range(8, TC):
        producers.append((("v", tt), v_gen(tt)))
    producers.append((("qT", 0, 2), qT_gen(0, 2)))
    producers.append((("kT", 2), kT_gen(2)))
    producers.append((("qT", 0, 3), qT_gen(0, 3)))
    producers.append((("kT", 3), kT_gen(3)))

    # ---- main attention loop ---------------------------------------------
    for st in range(ST):
        s0 = 512 * st
        for c in range(DC):
            # data this window consumes MUST be emitted before its matmuls
            if not (st == 0 and c == 0):
                pump_until(("qT", st, c))
                pump_until(("kT", c))
            cx = [
                cxps.tile([DA, 512], F32, tag="cx", name=f"cx{st}_{c}_{j}")
                for j in range(2)
            ]
            first = st == 0 and c == 0
            ex_tiles = {}

            def emit_sc(t):
                sc = scps.tile([128, 1024], F32, tag="sc")
                for j in range(2):
                    nc.tensor.matmul(
                        sc[:, 512 * j : 512 * (j + 1)],
                        lhsT=kT_sb[64 * j : 64 * (j + 1), c, 128 * t : 128 * (t + 1)],
                        rhs=qT_sb[64 * j : 64 * (j + 1), c, s0 : s0 + 512],
                        start=True,
                        stop=True,
                    )
                ex = expool.tile([128, 1024], MM_DT, tag="ex")
                nc.scalar.activation(
                    out=ex,
                    in_=sc,
                    func=mybir.ActivationFunctionType.Exp,
                    scale=float(SCALE),
                )
                ex_tiles[t] = ex

            def emit_cx(t):
                pump_until(("v", t))
                ex = ex_tiles.pop(t)
                for j in range(2):
                    nc.tensor.matmul(
                        cx[j],
                        lhsT=v_sb[:, t, 2 * c + j, :],
                        rhs=ex[:, 512 * j : 512 * (j + 1)],
                        start=(t == 0),
                        stop=(t == TC - 1),
                    )

            if first:
                # sc/exp stream ahead while v is still being produced
                for t in range(TC):
                    emit_sc(t)
                    pump(4)
                for t in range(TC):
                    emit_cx(t)
                    pump(1)
            else:
                for t in range(TC):
                    emit_sc(t)
                    emit_cx(t)
                    pump(2)

            # normalize: ctxT[64j:64j+64, c, s-tile] = cx[j][0:64] / cx[j][64]
            for j in range(2):
                rec = small.tile([1, 512], F32, tag="rec")
                nc.vector.reciprocal(rec, cx[j][D : D + 1, :])
                bc = small.tile([64, 512], F32, tag="tmp", name=f"bc{st}_{c}_{j}")
                nc.gpsimd.partition_broadcast(bc, rec)
                nc.vector.tensor_mul(
                    ctxT_sb[64 * j : 64 * (j + 1), c, s0 : s0 + 512],
                    cx[j][0:D, :],
                    bc,
                )

        # after the whole s-tile is normalized, queue its output projection
        if st + 1 < ST:
            producers.append((("qT", st + 1, 0), qT_gen(st + 1, 0)))
        for si in range(4):
            o_tile = osb.tile([128, E], F32, tag="osb", name=f"osb_{st}_{si}")
            producers.append((("fp", st, si), fp_gen(st, si, o_tile)))
        if st + 1 < ST:
            for c in range(1, DC):
                producers.append((("qT", st + 1, c), qT_gen(st + 1, c)))

    # ---- epilogue: drain remaining producers ------------------------------
    pump(10**9)


_BUILT = {}


def _get_nc():
    if "nc" not in _BUILT:
        _BUILT["nc"] = build_nc()
    return _BUILT["nc"]


def make_in_maps(query, key, value, Wq, Wk, Wv, Wo):
    ndt = mybir.dt.np(MM_DT)
    query = np.asarray(query, np.float32).astype(ndt)
    key = np.asarray(key, np.float32).astype(ndt)
    value = np.asarray(value, np.float32).astype(ndt)
    Wq = np.asarray(Wq, np.float32).astype(ndt)
    Wk = np.asarray(Wk, np.float32).astype(ndt)
    Wv = np.asarray(Wv, np.float32).astype(ndt)
    Wo = np.asarray(Wo, np.float32).astype(ndt)

    xqT = [np.ascontiguousarray(query[b].T) for b in range(B)]
    xkT = [np.ascontiguousarray(key[b].T) for b in range(B)]
    xvT = [np.ascontiguousarray(value[b].T) for b in range(B)]
    wqT = [np.ascontiguousarray(Wq[DH * g : DH * (g + 1), :].T) for g in range(HG)]
    wkT = [np.ascontiguousarray(Wk[DH * g : DH * (g + 1), :].T) for g in range(HG)]
    wvT = [np.ascontiguousarray(Wv[DH * g : DH * (g + 1), :].T) for g in range(HG)]
    woT = [np.ascontiguousarray(Wo[:, DH * g : DH * (g + 1)].T) for g in range(HG)]

    in_maps = []
    for core in range(NCORES):
        b, g = core // HG, core % HG
        in_maps.append(
            {
                "xqT": xqT[b],
                "xkT": xkT[b],
                "xvT": xvT[b],
                "wqT": wqT[g],
                "wkT": wkT[g],
                "wvT": wvT[g],
                "woT": woT[g],
            }
        )
    return in_maps


def assemble(core_outs):
    out = np.empty((B, S, E), np.float32)
    for b in range(B):
        out[b] = core_outs[HG * b]
        for g in range(1, HG):
            out[b] += core_outs[HG * b + g]
    return out


def kernel(query, key, value, Wq, Wk, Wv, Wo):
    from concourse.bass_utils import run_bass_kernel_spmd

    nc = _get_nc()
    in_maps = make_in_maps(query, key, value, Wq, Wk, Wv, Wo)
    res = run_bass_kernel_spmd(nc, in_maps, list(range(NCORES)))
    return assemble([r["out"] for r in res.results])


# revision 15
# speedup vs baseline: 1.3306x; 1.0191x over previous
"""Multi-head attention TRN2 kernel (v2, bf16 software-pipelined).

Sharding: 8 cores = 4 batches x 2 head-groups (Megatron tensor parallel over
the 16 heads: Wq/Wk/Wv column-sharded, Wo row-sharded; partial outputs summed
per batch on the host).

Per-core schedule (batch b, head-group g -> 8 local heads, 4 head-pairs c):
  prologue: kT(c=0) + qT(st=0,c=0) projections; v/kT(c>0)/qT interleave later
  main loop over (st, c): 16 t-chunks:
     scoresT[t,s] pair = kT_h.T @ qT_h      (K=64, j-pair co-executes on PE)
     ex = exp(scores/8) on ACT (PSUM->SBUF bf16)
     cx[65,s] += v_aug.T @ ex               (ones column -> softmax sums)
  producer mms (v proj, later kT/qT, out-proj) pumped into PE slack so the
  ACT engine (exp, ~283us total) stays saturated; normalize on DVE/Pool.
  out partial = ctxT.T @ WoT accumulated per 128-row chunk, DMA'd out.
"""

import os
import sys
from collections import deque
from contextlib import ExitStack

for _p in ("/opt/trn_rl_repo", "/root/.axon_site/_ro/trn_rl_repo"):
    if os.path.isdir(_p) and _p not in sys.path:
        sys.path.insert(0, _p)
        break

import numpy as np

import concourse.bass as bass
import concourse.bacc as bacc
import concourse.mybir as mybir
import concourse.tile as tile

B, S, E, H, D = 4, 2048, 1024, 16, 64
HG = 2          # head groups (tensor-parallel factor)
DH = E // HG    # 512 dims per head group (8 heads)
HPG = H // HG   # 8 heads per group
NCORES = B * HG

EC = E // 128   # 8 contraction chunks for projections
DC = DH // 128  # 4 d-chunks (head pairs)
TC = S // 128   # 16 t chunks
ST = S // 512   # 4 s tiles
SQ = S // 512   # 4 column blocks for projections
DA = D + 1      # 65: head dim + ones column

F32 = mybir.dt.float32
MM_DT = mybir.dt.bfloat16
SCALE = 1.0 / np.sqrt(D)


def build_nc():
    nc = bacc.Bacc()
    xqT = nc.declare_dram_parameter("xqT", [E, S], MM_DT, isOutput=False)
    xkT = nc.declare_dram_parameter("xkT", [E, S], MM_DT, isOutput=False)
    xvT = nc.declare_dram_parameter("xvT", [E, S], MM_DT, isOutput=False)
    wqT = nc.declare_dram_parameter("wqT", [E, DH], MM_DT, isOutput=False)
    wkT = nc.declare_dram_parameter("wkT", [E, DH], MM_DT, isOutput=False)
    wvT = nc.declare_dram_parameter("wvT", [E, DH], MM_DT, isOutput=False)
    woT = nc.declare_dram_parameter("woT", [DH, E], MM_DT, isOutput=False)
    out = nc.declare_dram_parameter("out", [S, E], F32, isOutput=True)

    with (
        nc.allow_low_precision(reason="bf16 matmul operands"),
        tile.TileContext(nc) as tc,
        ExitStack() as ctx,
    ):
        _emit(ctx, tc, xqT, xkT, xvT, wqT, wkT, wvT, woT, out)
    nc.compile()
    return nc


def _emit(ctx, tc, xqT, xkT, xvT, wqT, wkT, wvT, woT, out):
    nc = tc.nc

    big = ctx.enter_context(tc.tile_pool(name="big", bufs=1))
    # qT/kT/ctxT: [p, c, s] with local dim ld = 128*c + p
    # (head 2c on partitions 0-63, head 2c+1 on 64-127)
    qT_sb = big.tile([128, DC, S], MM_DT, tag="qT")
    kT_sb = big.tile([128, DC, S], MM_DT, tag="kT")
    ctxT_sb = big.tile([128, DC, S], MM_DT, tag="ctx")
    # v_aug: [t%128, t_chunk, head, 65]; col 64 is the ones column
    v_sb = big.tile([128, TC, HPG, DA], MM_DT, tag="v")
    wq_sb = big.tile([128, EC, DH], MM_DT, tag="wq")
    wk_sb = big.tile([128, EC, DH], MM_DT, tag="wk")
    wv_sb = big.tile([128, EC, DH], MM_DT, tag="wv")
    wo_sb = big.tile([128, DC, E], MM_DT, tag="wo")
    xk_st = big.tile([128, EC, S], MM_DT, tag="xk")
    xv_st = big.tile([128, EC, S], MM_DT, tag="xv")
    ones = big.tile([128, 1], MM_DT, tag="ones")
    nc.vector.memset(ones, 1.0)
    nc.vector.tensor_copy(
        v_sb[:, :, :, D : D + 1],
        ones.to_broadcast((128, TC * HPG)).rearrange(
            "p (t h o) -> p t h o", t=TC, h=HPG
        ),
    )

    xqp = ctx.enter_context(tc.tile_pool(name="xqp", bufs=2))
    expool = ctx.enter_context(tc.tile_pool(name="ex", bufs=5))
    osb = ctx.enter_context(tc.tile_pool(name="osb", bufs=2))
    small = ctx.enter_context(tc.tile_pool(name="small", bufs=2))
    scps = ctx.enter_context(tc.tile_pool(name="scps", bufs=2, space="PSUM"))
    cxps = ctx.enter_context(tc.tile_pool(name="cxps", bufs=2, space="PSUM"))
    accps = ctx.enter_context(tc.tile_pool(name="accps", bufs=2, space="PSUM"))

    # ---- DMA helpers: alternate the two DMA-capable idle engines ----------
    dma_state = [0]

    def dma(out_ap, in_ap):
        eng = nc.sync if dma_state[0] % 2 == 0 else nc.gpsimd
        dma_state[0] += 1
        eng.dma_start(out=out_ap, in_=in_ap)

    # priority-ordered loads: what the prologue needs first
    for e in range(EC):
        dma(wk_sb[:, e, :], wkT[128 * e : 128 * (e + 1), :])
    for e in range(EC):
        dma(xk_st[:, e, 0:512], xkT[128 * e : 128 * (e + 1), 0:512])
    for e in range(EC):
        dma(wq_sb[:, e, :], wqT[128 * e : 128 * (e + 1), :])
    xq_tiles = {}
    xq_tiles[0] = xqp.tile([128, EC, 512], MM_DT, tag="xq", name="xq_st0")
    for e in range(EC):
        dma(xq_tiles[0][:, e, :], xqT[128 * e : 128 * (e + 1), 0:512])
    for sq in range(1, SQ):
        for e in range(EC):
            dma(
                xk_st[:, e, 512 * sq : 512 * (sq + 1)],
                xkT[128 * e : 128 * (e + 1), 512 * sq : 512 * (sq + 1)],
            )
    for e in range(EC):
        dma(wv_sb[:, e, :], wvT[128 * e : 128 * (e + 1), :])
    for sq in range(SQ):
        for e in range(EC):
            dma(
                xv_st[:, e, 512 * sq : 512 * (sq + 1)],
                xvT[128 * e : 128 * (e + 1), 512 * sq : 512 * (sq + 1)],
            )
    for a in range(DC):
        dma(wo_sb[:, a, :], woT[128 * a : 128 * (a + 1), :])

    # ---- producer generators (yield after each matmul) --------------------
    def kT_gen(c):
        for sq in range(SQ):
            acc = accps.tile([128, 512], F32, tag="acc", name=f"kacc_{c}_{sq}")
            for e in range(EC):
                nc.tensor.matmul(
                    acc,
                    lhsT=wk_sb[:, e, 128 * c : 128 * (c + 1)],
                    rhs=xk_st[:, e, 512 * sq : 512 * (sq + 1)],
                    start=(e == 0),
                    stop=(e == EC - 1),
                )
                yield
            nc.vector.tensor_copy(kT_sb[:, c, 512 * sq : 512 * (sq + 1)], acc)

    def v_gen(tt):
        acc = accps.tile([128, 512], F32, tag="acc", name=f"vacc_{tt}")
        for e in range(EC):
            nc.tensor.matmul(
                acc,
                lhsT=xv_st[:, e, 128 * tt : 128 * (tt + 1)],
                rhs=wv_sb[:, e, :],
                start=(e == 0),
                stop=(e == EC - 1),
            )
            yield
        nc.vector.tensor_copy(
            v_sb[:, tt, :, 0:D], acc.rearrange("p (h d) -> p h d", h=HPG)
        )

    def qT_gen(st, c):
        xq = xq_tiles[st]
        acc = accps.tile([128, 512], F32, tag="acc", name=f"qacc_{st}_{c}")
        for e in range(EC):
            nc.tensor.matmul(
                acc,
                lhsT=wq_sb[:, e, 128 * c : 128 * (c + 1)],
                rhs=xq[:, e, :],
                start=(e == 0),
                stop=(e == EC - 1),
            )
            yield
        nc.vector.tensor_copy(qT_sb[:, c, 512 * st : 512 * (st + 1)], acc)
        if c == DC - 1 and st + 1 < ST:
            # stage next s-tile's xq chunks
            nxt = xqp.tile([128, EC, 512], MM_DT, tag="xq", name=f"xq_st{st+1}")
            xq_tiles[st + 1] = nxt
            for e in range(EC):
                dma(
                    nxt[:, e, :],
                    xqT[128 * e : 128 * (e + 1), 512 * (st + 1) : 512 * (st + 2)],
                )

    def fp_gen(st, si, o_tile):
        r0 = 512 * st + 128 * si
        for et in range(2):
            fp = accps.tile([128, 512], F32, tag="acc", name=f"fp_{r0}_{et}")
            for cc in range(DC):
                nc.tensor.matmul(
                    fp,
                    lhsT=ctxT_sb[:, cc, r0 : r0 + 128],
                    rhs=wo_sb[:, cc, 512 * et : 512 * (et + 1)],
                    start=(cc == 0),
                    stop=(cc == DC - 1),
                )
                yield
            nc.vector.tensor_copy(o_tile[:, 512 * et : 512 * (et + 1)], fp)
        for p0 in range(0, 128, 32):
            dma(out[r0 + p0 : r0 + p0 + 32, :], o_tile[p0 : p0 + 32, :])

    # producer queue machinery: (key, generator) FIFO with forced drains
    producers = deque()
    done_keys = set()
    cur = [None, None]  # key, generator

    def _finish_cur():
        done_keys.add(cur[0])
        cur[0] = cur[1] = None

    def pump(n):
        emitted = 0
        while emitted < n:
            if cur[1] is None:
                if not producers:
                    return
                cur[0], cur[1] = producers.popleft()
            try:
                next(cur[1])
                emitted += 1
            except StopIteration:
                _finish_cur()

    def pump_until(key):
        while key not in done_keys:
            if cur[1] is None:
                if not producers:
                    raise RuntimeError(f"producer underflow waiting for {key}")
                cur[0], cur[1] = producers.popleft()
            try:
                while True:
                    next(cur[1])
            except StopIteration:
                _finish_cur()

    # ---- prologue ---------------------------------------------------------
    for _ in kT_gen(0):
        pass
    for _ in qT_gen(0, 0):
        pass
    done_keys.add(("kT", 0))
    done_keys.add(("qT", 0, 0))

    # initial producer order: v first (needed by cxmm), then kT/qT for the
    # upcoming head-pairs / s-tiles
    for tt in range(8):
        producers.append((("v", tt), v_gen(tt)))
    producers.append((("qT", 0, 1), qT_gen(0, 1)))
    producers.append((("kT", 1), kT_gen(1)))
    for tt in range(8, TC):
        producers.append((("v", tt), v_gen(tt)))
    producers.append((("qT", 0, 2), qT_gen(0, 2)))
    producers.append((("kT", 2), kT_gen(2)))
    producers.append((("qT", 0, 3), qT_gen(0, 3)))
    producers.append((("kT", 3), kT_gen(3)))

    # ---- main attention loop ---------------------------------------------
    for st in range(ST):
        s0 = 512 * st
        for c in range(DC):
            # data this window consumes MUST be emitted before its matmuls
            if not (st == 0 and c == 0):
                pump_until(("qT", st, c))
                pump_until(("kT", c))
            cx = [
                cxps.tile([DA, 512], F32, tag="cx", name=f"cx{st}_{c}_{j}")
                for j in range(2)
            ]
            first = st == 0 and c == 0
            ex_tiles = {}

            def emit_sc(t):
                sc = scps.tile([128, 1024], F32, tag="sc")
                for j in range(2):
                    nc.tensor.matmul(
                        sc[:, 512 * j : 512 * (j + 1)],
                        lhsT=kT_sb[64 * j : 64 * (j + 1), c, 128 * t : 128 * (t + 1)],
                        rhs=qT_sb[64 * j : 64 * (j + 1), c, s0 : s0 + 512],
                        start=True,
                        stop=True,
                    )
                ex = expool.tile([128, 1024], MM_DT, tag="ex")
                nc.scalar.activation(
                    out=ex,
                    in_=sc,
                    func=mybir.ActivationFunctionType.Exp,
                    scale=float(SCALE),
                )
                ex_tiles[t] = ex

            def emit_cx(t):
                pump_until(("v", t))
                ex = ex_tiles.pop(t)
                for j in range(2):
                    nc.tensor.matmul(
                        cx[j],
                        lhsT=v_sb[:, t, 2 * c + j, :],
                        rhs=ex[:, 512 * j : 512 * (j + 1)],
                        start=(t == 0),
                        stop=(t == TC - 1),
                    )

            if first:
                # sc/exp stream ahead while v is still being produced
                for t in range(TC):
                    emit_sc(t)
                    pump(4)
                for t in range(TC):
                    emit_cx(t)
                    pump(1)
            else:
                for t in range(TC):
                    emit_sc(t)
                    emit_cx(t)
                    pump(2)

            # normalize: ctxT[64j:64j+64, c, s-tile] = cx[j][0:64] / cx[j][64]
            # copy PSUM->SBUF first (fast, on Pool) so the cx bank frees
            # immediately; the slow reciprocal runs off the critical path
            cxs = []
            for j in range(2):
                t_ = small.tile([DA, 512], F32, tag="cxs", name=f"cxs{st}_{c}_{j}")
                nc.vector.tensor_copy(t_, cx[j])
                cxs.append(t_)
            for j in range(2):
                rec = small.tile([1, 512], F32, tag="rec")
                nc.vector.reciprocal(rec, cxs[j][D : D + 1, :])
                bc = small.tile([64, 512], F32, tag="tmp", name=f"bc{st}_{c}_{j}")
                nc.gpsimd.partition_broadcast(bc, rec)
                nc.vector.tensor_mul(
                    ctxT_sb[64 * j : 64 * (j + 1), c, s0 : s0 + 512],
                    cxs[j][0:D, :],
                    bc,
                )

        # after the whole s-tile is normalized, queue its output projection
        if st + 1 < ST:
            producers.append((("qT", st + 1, 0), qT_gen(st + 1, 0)))
        for si in range(4):
            o_tile = osb.tile([128, E], F32, tag="osb", name=f"osb_{st}_{si}")
            producers.append((("fp", st, si), fp_gen(st, si, o_tile)))
        if st + 1 < ST:
            for c in range(1, DC):
                producers.append((("qT", st + 1, c), qT_gen(st + 1, c)))

    # ---- epilogue: drain remaining producers ------------------------------
    pump(10**9)


_BUILT = {}


def _get_nc():
    if "nc" not in _BUILT:
        _BUILT["nc"] = build_nc()
    return _BUILT["nc"]


def make_in_maps(query, key, value, Wq, Wk, Wv, Wo):
    ndt = mybir.dt.np(MM_DT)
    query = np.asarray(query, np.float32).astype(ndt)
    key = np.asarray(key, np.float32).astype(ndt)
    value = np.asarray(value, np.float32).astype(ndt)
    Wq = np.asarray(Wq, np.float32).astype(ndt)
    Wk = np.asarray(Wk, np.float32).astype(ndt)
    Wv = np.asarray(Wv, np.float32).astype(ndt)
    Wo = np.asarray(Wo, np.float32).astype(ndt)

    xqT = [np.ascontiguousarray(query[b].T) for b in range(B)]
    xkT = [np.ascontiguousarray(key[b].T) for b in range(B)]
    xvT = [np.ascontiguousarray(value[b].T) for b in range(B)]
    wqT = [np.ascontiguousarray(Wq[DH * g : DH * (g + 1), :].T) for g in range(HG)]
    wkT = [np.ascontiguousarray(Wk[DH * g : DH * (g + 1), :].T) for g in range(HG)]
    wvT = [np.ascontiguousarray(Wv[DH * g : DH * (g + 1), :].T) for g in range(HG)]
    woT = [np.ascontiguousarray(Wo[:, DH * g : DH * (g + 1)].T) for g in range(HG)]

    in_maps = []
    for core in range(NCORES):
        b, g = core // HG, core % HG
        in_maps.append(
            {
                "xqT": xqT[b],
                "xkT": xkT[b],
                "xvT": xvT[b],
                "wqT": wqT[g],
                "wkT": wkT[g],
                "wvT": wvT[g],
                "woT": woT[g],
            }
        )
    return in_maps


def assemble(core_outs):
    out = np.empty((B, S, E), np.float32)
    for b in range(B):
        out[b] = core_outs[HG * b]
        for g in range(1, HG):
            out[b] += core_outs[HG * b + g]
    return out


def kernel(query, key, value, Wq, Wk, Wv, Wo):
    from concourse.bass_utils import run_bass_kernel_spmd

    nc = _get_nc()
    in_maps = make_in_maps(query, key, value, Wq, Wk, Wv, Wo)
    res = run_bass_kernel_spmd(nc, in_maps, list(range(NCORES)))
    return assemble([r["out"] for r in res.results])
